# revision 6
# baseline (speedup 1.0000x reference)
"""GQA kernel for 8 trn2 NeuronCores (v2, bf16 + DMA-transpose).

Problem: B=2, T=2048, E=2048, G=16 q-heads, H=4 kv-heads, D=128.
Sharding: core c -> batch b=c//4, head-group g=c%4 (query heads 4g..4g+3,
which all share kv head g). Each core computes a [T, E] partial of the
output projection (contraction over its 512 head-channels of Wo); the
host sums the 4 partials per batch.

v2 changes vs baseline (546 us):
  - All inputs converted to bf16 on the host; X^T obtained directly via
    dma_start_transpose (xbar DMA transpose, bf16) instead of PE
    transposes + DVE copies. Kills ~131k PE cycles and ~8.4M DVE elems,
    and halves X DMA bytes.
  - All matmuls bf16 (1 cyc/row like f32r, half the SBUF/ldweights
    traffic).
  - Softmax sums off the PE: gpsimd accumulates exp tiles into a [128,
    QCH] fp32 acc; a single [128->1] ones-matmul per (head, q-chunk)
    finishes the partition reduction (3.4 us PE total vs 54.6 us for
    the per-tile ones-matmul).
  - Normalization (recip -> partition_broadcast -> A^T mul) is
    software-pipelined one (head, q-chunk) late so no engine stalls on
    the cross-engine chain.
  - Output written bf16 (halves write traffic); host sums partials in
    fp32.
Per-core dataflow:
  X^T (dma transpose) -> K^T, V^T (+V natural via PE transpose), Q^T
  S^T[k,q] = K^T-tile stationary x Q^T moving ; P^T = exp(S^T/sqrt(D))
  O^T[d,q] += V-tile stationary x P^T moving   (psum accum over k)
  acc += P^T tiles (gpsimd) ; Z = ones^T x acc ; A^T = O^T * (1/Z)
  out[t,e] = sum_n A^T[n,t] Wo[n,e]  (interleaved with attention)
The all-True mask input is ignored.
"""

import contextlib

import ml_dtypes
import numpy as np

import concourse.bass as bass
import concourse.tile as tile
from concourse import bacc, mybir
from concourse.bass_utils import run_bass_kernel_spmd
from concourse.masks import make_identity

T = 2048
E = 2048
NH = 4          # query heads per core
D = 128
ND = NH * D     # 512 local projection width
GCH = 1024      # token group for transposed X loads
NG = T // GCH   # 2
QCH = 512       # query/token chunk for compute phases
NSUB = GCH // QCH  # 2
NQC = T // QCH  # 4
NKT = T // 128  # 16 key tiles
NET = E // 128  # 16 e tiles
SCALE = float(1.0 / np.sqrt(D))

FP32 = mybir.dt.float32
F32R = mybir.dt.float32r
BF16 = mybir.dt.bfloat16


def _build_core_program():
    nc = bacc.Bacc(
        "TRN2", target_bir_lowering=False, debug=False, enable_asserts=False
    )
    xq = nc.dram_tensor("xq", [T, E], BF16, kind="ExternalInput").ap()
    xkv = nc.dram_tensor("xkv", [T, E], BF16, kind="ExternalInput").ap()
    wq = nc.dram_tensor("wq", [E, ND], BF16, kind="ExternalInput").ap()
    wk = nc.dram_tensor("wk", [E, D], BF16, kind="ExternalInput").ap()
    wv = nc.dram_tensor("wv", [E, D], BF16, kind="ExternalInput").ap()
    wo = nc.dram_tensor("wo", [ND, E], BF16, kind="ExternalInput").ap()
    out = nc.dram_tensor("out", [T, E], BF16, kind="ExternalOutput").ap()

    with tile.TileContext(nc) as tc:
        _body(tc, xq, xkv, wq, wk, wv, wo, out)
    nc.compile()
    return nc


def _body(tc, xq, xkv, wq, wk, wv, wo, out):
    nc = tc.nc
    exp = mybir.ActivationFunctionType.Exp

    with contextlib.ExitStack() as ctx:
        consts = ctx.enter_context(tc.tile_pool(name="consts", bufs=1))
        persist = ctx.enter_context(tc.tile_pool(name="persist", bufs=1))
        wpool = ctx.enter_context(tc.tile_pool(name="weights", bufs=1))
        xtp = ctx.enter_context(tc.tile_pool(name="xt", bufs=2))
        sbwork = ctx.enter_context(tc.tile_pool(name="work", bufs=2))
        ptpool = ctx.enter_context(tc.tile_pool(name="ptp", bufs=6))
        outpool = ctx.enter_context(tc.tile_pool(name="outstage", bufs=4))
        pall = ctx.enter_context(
            tc.tile_pool(name="pall", bufs=1, space="PSUM")
        )

        ident = consts.tile([128, 128], BF16)
        make_identity(nc, ident[:])
        ones_f32 = consts.tile([128, 1], FP32)
        nc.vector.memset(ones_f32[:], 1.0)
        ones_fr = consts.tile([128, 1], F32R)
        nc.vector.tensor_copy(ones_fr[:], ones_f32[:])

        # persistent bf16 operands
        kT = persist.tile([128, T], BF16)              # K^T  [d, t]
        vN = persist.tile([128, NKT, D], BF16)         # V natural [t, d] tiles
        qT = persist.tile([128, NH, T], BF16)          # Q^T  [n, t]
        aTq = [
            persist.tile([128, NH, QCH], BF16, name=f"aT{i}")
            for i in range(NQC)
        ]

        # weights land directly as bf16 (no staging copies)
        wk_sb = wpool.tile([128, NET, D], BF16)
        wv_sb = wpool.tile([128, NET, D], BF16)
        wq_sb = wpool.tile([128, NET, ND], BF16)
        wo_sb = wpool.tile([128, NH, E], BF16)

        nc.scalar.dma_start(wk_sb[:], wk.rearrange("(a p) d -> p a d", p=128))
        nc.scalar.dma_start(wv_sb[:], wv.rearrange("(a p) d -> p a d", p=128))

        def load_xt_group(src, g):
            """Transposed-DMA a [GCH, E] row-group of src into 16 bf16
            tiles xts[et] = X^T[et*128:(et+1)*128, g*GCH:(g+1)*GCH]."""
            xts = []
            for et in range(NET):
                t = xtp.tile([128, GCH], BF16, tag=f"xt{et}")
                # NB: concurrent transpose-DMAs from two HWDGE queues corrupt
                # each other (shared xbar) — keep them all on the sync queue.
                nc.sync.dma_start_transpose(
                    t[:], src[g * GCH : (g + 1) * GCH, et * 128 : (et + 1) * 128]
                )
                xts.append(t)
            return xts

        # ---- phase 1: Xkv -> K^T, V natural ----
        first_kv = []
        for g in range(NG):
            xts = load_xt_group(xkv, g)
            if g == 0:
                # big weights overlap the first KV compute
                nc.scalar.dma_start(
                    wq_sb[:], wq.rearrange("(a p) n -> p a n", p=128)
                )
                nc.scalar.dma_start(
                    wo_sb[:], wo.rearrange("(a p) e -> p a e", p=128)
                )
            for s in range(NSUB):
                t0 = g * GCH + s * QCH
                ms = slice(s * QCH, (s + 1) * QCH)
                kp = pall.tile([128, QCH], FP32, tag="st", bufs=3)
                for et in range(NET):
                    nc.tensor.matmul(
                        kp[:], wk_sb[:, et, :], xts[et][:, ms],
                        start=(et == 0), stop=(et == NET - 1),
                    )
                nc.vector.tensor_copy(kT[:, t0 : t0 + QCH], kp[:])
                vp = pall.tile([128, QCH], FP32, tag="st", bufs=3)
                for et in range(NET):
                    nc.tensor.matmul(
                        vp[:], wv_sb[:, et, :], xts[et][:, ms],
                        start=(et == 0), stop=(et == NET - 1),
                    )
                vtb = sbwork.tile([128, QCH], BF16, tag="vtb")
                nc.vector.tensor_copy(vtb[:], vp[:])
                vnp = pall.tile([128, QCH // 128, 128], BF16, tag="wo", bufs=2)
                for i in range(QCH // 128):
                    nc.tensor.transpose(
                        vnp[:, i, :], vtb[:, i * 128 : (i + 1) * 128], ident[:]
                    )
                nc.vector.tensor_copy(
                    vN[:, t0 // 128 : t0 // 128 + QCH // 128, :], vnp[:]
                )

        # ---- phase 2: Xq -> Q^T ----
        for g in range(NG):
            xts = load_xt_group(xq, g)
            for s in range(NSUB):
                t0 = g * GCH + s * QCH
                for nt in range(NH):
                    qp = pall.tile([128, QCH], FP32, tag="st", bufs=3)
                    for et in range(NET):
                        nc.tensor.matmul(
                            qp[:],
                            wq_sb[:, et, nt * 128 : (nt + 1) * 128],
                            xts[et][:, s * QCH : (s + 1) * QCH],
                            start=(et == 0), stop=(et == NET - 1),
                        )
                    nc.vector.tensor_copy(qT[:, nt, t0 : t0 + QCH], qp[:])

        # ---- phase 3+4: attention; output projection and the softmax
        # normalization chain are software-pipelined into later
        # iterations so the PE stream never waits on other engines ----
        wo_pending = []   # (tt, ec) tiles whose aT inputs are ready
        wo_state = {"cur": None, "wp": None, "nt": 0}
        norm_pending = []  # deferred (qc, h, op, sm) normalizations

        def wo_step():
            stt = wo_state
            if stt["cur"] is None:
                if not wo_pending:
                    return
                stt["cur"] = wo_pending.pop(0)
                stt["wp"] = pall.tile(
                    [128, QCH], FP32, tag="wo", bufs=2, name="wp"
                )
                stt["nt"] = 0
            tt, ec = stt["cur"]
            nt = stt["nt"]
            nc.tensor.matmul(
                stt["wp"][:],
                aTq[tt // 4][:, nt, (tt % 4) * 128 : (tt % 4 + 1) * 128],
                wo_sb[:, nt, ec * QCH : (ec + 1) * QCH],
                start=(nt == 0), stop=(nt == NH - 1),
            )
            stt["nt"] += 1
            if stt["nt"] == NH:
                ob = outpool.tile([128, QCH], BF16, tag="ob", name="ob")
                nc.vector.tensor_copy(ob[:], stt["wp"][:])
                nc.sync.dma_start(
                    out[tt * 128 : (tt + 1) * 128,
                        ec * QCH : (ec + 1) * QCH],
                    ob[:],
                )
                stt["cur"] = None

        def finish_norm():
            if not norm_pending:
                return
            qc0, h0, op0, sm0 = norm_pending.pop(0)
            rb = sbwork.tile([128, QCH], FP32, tag="rb")
            nc.gpsimd.partition_broadcast(rb[:], sm0[:])
            nc.vector.tensor_mul(aTq[qc0][:, h0, :], op0[:], rb[:])
            if h0 == NH - 1:
                wo_pending.extend(
                    (tt, ec)
                    for tt in range(qc0 * 4, (qc0 + 1) * 4)
                    for ec in range(E // QCH)
                )

        for qc in range(NQC):
            qs = slice(qc * QCH, (qc + 1) * QCH)
            for h in range(NH):
                op = pall.tile([128, QCH], FP32, tag="ot", bufs=2)
                acc = sbwork.tile([128, QCH], F32R, tag="acc")

                DEPTH = 3
                pts = [None] * NKT

                def issue_scores(kt):
                    st = pall.tile(
                        [128, QCH], FP32, tag="st", bufs=3, name="st"
                    )
                    nc.tensor.matmul(
                        st[:],
                        kT[:, kt * 128 : (kt + 1) * 128],
                        qT[:, h, qs],
                        start=True, stop=True,
                    )
                    pt = ptpool.tile([128, QCH], BF16, tag="pt", name="pt")
                    nc.scalar.activation(pt[:], st[:], exp, scale=SCALE)
                    if kt == 0:
                        nc.gpsimd.tensor_copy(acc[:], pt[:])
                    else:
                        nc.gpsimd.tensor_add(acc[:], acc[:], pt[:])
                    pts[kt] = pt

                for kt in range(DEPTH):
                    issue_scores(kt)
                    wo_step()
                for kt in range(NKT):
                    if kt + DEPTH < NKT:
                        issue_scores(kt + DEPTH)
                    if kt == 2:
                        finish_norm()
                    nc.tensor.matmul(
                        op[:], vN[:, kt, :], pts[kt][:],
                        start=(kt == 0), stop=(kt == NKT - 1),
                    )
                    wo_step()
                # partition-reduce acc -> Z, then defer the rest
                sp = pall.tile([1, QCH], FP32, tag="sm", bufs=1, name="sp")
                nc.tensor.matmul(
                    sp[:], ones_fr[:], acc[:], start=True, stop=True,
                )
                sm = sbwork.tile([1, QCH], FP32, tag="sm2")
                nc.vector.reciprocal(sm[:], sp[:])
                norm_pending.append((qc, h, op, sm))

        finish_norm()
        while wo_pending or wo_state["cur"] is not None:
            wo_step()


_NC_CACHE = []


def _get_nc():
    if not _NC_CACHE:
        _NC_CACHE.append(_build_core_program())
    return _NC_CACHE[0]


def _make_in_maps(inputs_q, inputs_kv, Wq, Wk, Wv, Wo):
    bf = lambda a: np.ascontiguousarray(a).astype(ml_dtypes.bfloat16)
    in_maps = []
    for core in range(8):
        b, g = core // 4, core % 4
        in_maps.append(
            {
                "xq": bf(inputs_q[b]),
                "xkv": bf(inputs_kv[b]),
                "wq": bf(Wq[:, g * ND : (g + 1) * ND]),
                "wk": bf(Wk[:, g * D : (g + 1) * D]),
                "wv": bf(Wv[:, g * D : (g + 1) * D]),
                "wo": bf(Wo[g * ND : (g + 1) * ND, :]),
            }
        )
    return in_maps


def _run(inputs_q, inputs_kv, Wq, Wk, Wv, Wo, trace=False, **trace_kwargs):
    nc = _get_nc()
    in_maps = _make_in_maps(inputs_q, inputs_kv, Wq, Wk, Wv, Wo)
    res = run_bass_kernel_spmd(
        nc, in_maps, core_ids=list(range(8)), trace=trace, **trace_kwargs
    )
    parts = [np.asarray(r["out"], dtype=np.float32) for r in res.results]
    full = np.stack(
        [
            parts[0] + parts[1] + parts[2] + parts[3],
            parts[4] + parts[5] + parts[6] + parts[7],
        ]
    ).astype(np.float32)
    return full, res


def kernel(inputs_q, inputs_kv, Wq, Wk, Wv, Wo, mask=None):
    inputs_q = np.asarray(inputs_q, dtype=np.float32)
    inputs_kv = np.asarray(inputs_kv, dtype=np.float32)
    Wq = np.asarray(Wq, dtype=np.float32)
    Wk = np.asarray(Wk, dtype=np.float32)
    Wv = np.asarray(Wv, dtype=np.float32)
    Wo = np.asarray(Wo, dtype=np.float32)
    full, _ = _run(inputs_q, inputs_kv, Wq, Wk, Wv, Wo, trace=False)
    return full


# revision 9
# speedup vs baseline: 1.7609x; 1.7609x over previous
"""GQA kernel for 8 trn2 NeuronCores (v2, bf16 + DMA-transpose).

Problem: B=2, T=2048, E=2048, G=16 q-heads, H=4 kv-heads, D=128.
Sharding: core c -> batch b=c//4, head-group g=c%4 (query heads 4g..4g+3,
which all share kv head g). Each core computes a [T, E] partial of the
output projection (contraction over its 512 head-channels of Wo); the
host sums the 4 partials per batch.

v2 changes vs baseline (546 us):
  - All inputs converted to bf16 on the host; X^T obtained directly via
    dma_start_transpose (xbar DMA transpose, bf16) instead of PE
    transposes + DVE copies. Kills ~131k PE cycles and ~8.4M DVE elems,
    and halves X DMA bytes.
  - All matmuls bf16 (1 cyc/row like f32r, half the SBUF/ldweights
    traffic).
  - Softmax sums off the PE: gpsimd accumulates exp tiles into a [128,
    QCH] fp32 acc; a single [128->1] ones-matmul per (head, q-chunk)
    finishes the partition reduction (3.4 us PE total vs 54.6 us for
    the per-tile ones-matmul).
  - Normalization (recip -> partition_broadcast -> A^T mul) is
    software-pipelined one (head, q-chunk) late so no engine stalls on
    the cross-engine chain.
  - Output written bf16 (halves write traffic); host sums partials in
    fp32.
Per-core dataflow:
  X^T (dma transpose) -> K^T, V^T (+V natural via PE transpose), Q^T
  S^T[k,q] = K^T-tile stationary x Q^T moving ; P^T = exp(S^T/sqrt(D))
  O^T[d,q] += V-tile stationary x P^T moving   (psum accum over k)
  acc += P^T tiles (gpsimd) ; Z = ones^T x acc ; A^T = O^T * (1/Z)
  out[t,e] = sum_n A^T[n,t] Wo[n,e]  (interleaved with attention)
The all-True mask input is ignored.
"""

import contextlib

import ml_dtypes
import numpy as np

import concourse.bass as bass
import concourse.tile as tile
from concourse import bacc, mybir
from concourse.bass_utils import run_bass_kernel_spmd
from concourse.masks import make_identity

T = 2048
E = 2048
NH = 4          # query heads per core
D = 128
ND = NH * D     # 512 local projection width
GCH = 1024      # token group for transposed X loads
NG = T // GCH   # 2
QCH = 512       # query/token chunk for compute phases
NSUB = GCH // QCH  # 2
NQC = T // QCH  # 4
NKT = T // 128  # 16 key tiles
NET = E // 128  # 16 e tiles
SCALE = float(1.0 / np.sqrt(D))

FP32 = mybir.dt.float32
F32R = mybir.dt.float32r
BF16 = mybir.dt.bfloat16


def _build_core_program():
    nc = bacc.Bacc(
        "TRN2", target_bir_lowering=False, debug=False, enable_asserts=False
    )
    xq = nc.dram_tensor("xq", [T, E], BF16, kind="ExternalInput").ap()
    xkv = nc.dram_tensor("xkv", [T, E], BF16, kind="ExternalInput").ap()
    wq = nc.dram_tensor("wq", [E, ND], BF16, kind="ExternalInput").ap()
    wk = nc.dram_tensor("wk", [E, D], BF16, kind="ExternalInput").ap()
    wv = nc.dram_tensor("wv", [E, D], BF16, kind="ExternalInput").ap()
    wo = nc.dram_tensor("wo", [ND, E], BF16, kind="ExternalInput").ap()
    out = nc.dram_tensor("out", [T, E], BF16, kind="ExternalOutput").ap()

    with tile.TileContext(nc) as tc:
        _body(tc, xq, xkv, wq, wk, wv, wo, out)
    nc.compile()
    return nc


def _body(tc, xq, xkv, wq, wk, wv, wo, out):
    nc = tc.nc
    exp = mybir.ActivationFunctionType.Exp

    with contextlib.ExitStack() as ctx:
        consts = ctx.enter_context(tc.tile_pool(name="consts", bufs=1))
        persist = ctx.enter_context(tc.tile_pool(name="persist", bufs=1))
        wpool = ctx.enter_context(tc.tile_pool(name="weights", bufs=1))
        xtp = ctx.enter_context(tc.tile_pool(name="xt", bufs=2))
        sbwork = ctx.enter_context(tc.tile_pool(name="work", bufs=2))
        ptpool = ctx.enter_context(tc.tile_pool(name="ptp", bufs=6))
        outpool = ctx.enter_context(tc.tile_pool(name="outstage", bufs=4))
        pall = ctx.enter_context(
            tc.tile_pool(name="pall", bufs=1, space="PSUM")
        )

        ident = consts.tile([128, 128], BF16)
        make_identity(nc, ident[:])
        ones_bf = consts.tile([128, 1], BF16)
        nc.vector.memset(ones_bf[:], 1.0)

        # persistent bf16 operands
        kT = persist.tile([128, T], BF16)              # K^T  [d, t]
        vN = persist.tile([128, NKT, D], BF16)         # V natural [t, d] tiles
        qT = persist.tile([128, NH, T], BF16)          # Q^T  [n, t]
        aTq = [
            persist.tile([128, NH, QCH], BF16, name=f"aT{i}")
            for i in range(NQC)
        ]

        # weights land directly as bf16 (no staging copies)
        wk_sb = wpool.tile([128, NET, D], BF16)
        wv_sb = wpool.tile([128, NET, D], BF16)
        wq_sb = wpool.tile([128, NET, ND], BF16)
        wo_sb = wpool.tile([128, NH, E], BF16)

        nc.scalar.dma_start(wk_sb[:], wk.rearrange("(a p) d -> p a d", p=128))
        nc.scalar.dma_start(wv_sb[:], wv.rearrange("(a p) d -> p a d", p=128))

        def load_xt_group(src, g):
            """Transposed-DMA a [GCH, E] row-group of src into 16 bf16
            tiles xts[et] = X^T[et*128:(et+1)*128, g*GCH:(g+1)*GCH]."""
            xts = []
            for et in range(NET):
                t = xtp.tile([128, GCH], BF16, tag=f"xt{et}")
                # NB: concurrent transpose-DMAs from two HWDGE queues corrupt
                # each other (shared xbar) — keep them all on the sync queue.
                nc.sync.dma_start_transpose(
                    t[:], src[g * GCH : (g + 1) * GCH, et * 128 : (et + 1) * 128]
                )
                xts.append(t)
            return xts

        # ---- phase 1: Xkv -> K^T, V natural ----
        first_kv = []
        for g in range(NG):
            xts = load_xt_group(xkv, g)
            if g == 0:
                # big weights overlap the first KV compute
                nc.scalar.dma_start(
                    wq_sb[:], wq.rearrange("(a p) n -> p a n", p=128)
                )
                nc.scalar.dma_start(
                    wo_sb[:], wo.rearrange("(a p) e -> p a e", p=128)
                )
            for s in range(NSUB):
                t0 = g * GCH + s * QCH
                ms = slice(s * QCH, (s + 1) * QCH)
                kp = pall.tile([128, QCH], FP32, tag="st", bufs=3)
                for et in range(NET):
                    nc.tensor.matmul(
                        kp[:], wk_sb[:, et, :], xts[et][:, ms],
                        start=(et == 0), stop=(et == NET - 1),
                    )
                nc.vector.tensor_copy(kT[:, t0 : t0 + QCH], kp[:])
                vp = pall.tile([128, QCH], FP32, tag="st", bufs=3)
                for et in range(NET):
                    nc.tensor.matmul(
                        vp[:], wv_sb[:, et, :], xts[et][:, ms],
                        start=(et == 0), stop=(et == NET - 1),
                    )
                vtb = sbwork.tile([128, QCH], BF16, tag="vtb")
                nc.vector.tensor_copy(vtb[:], vp[:])
                vnp = pall.tile([128, QCH // 128, 128], BF16, tag="wo", bufs=2)
                for i in range(QCH // 128):
                    nc.tensor.transpose(
                        vnp[:, i, :], vtb[:, i * 128 : (i + 1) * 128], ident[:]
                    )
                nc.vector.tensor_copy(
                    vN[:, t0 // 128 : t0 // 128 + QCH // 128, :], vnp[:]
                )

        # ---- phase 2: Xq -> Q^T ----
        for g in range(NG):
            xts = load_xt_group(xq, g)
            for s in range(NSUB):
                t0 = g * GCH + s * QCH
                for nt in range(NH):
                    qp = pall.tile([128, QCH], FP32, tag="st", bufs=3)
                    for et in range(NET):
                        nc.tensor.matmul(
                            qp[:],
                            wq_sb[:, et, nt * 128 : (nt + 1) * 128],
                            xts[et][:, s * QCH : (s + 1) * QCH],
                            start=(et == 0), stop=(et == NET - 1),
                        )
                    nc.vector.tensor_copy(qT[:, nt, t0 : t0 + QCH], qp[:])

        # ---- phase 3+4: attention; output projection and the softmax
        # normalization chain are software-pipelined into later
        # iterations so the PE stream never waits on other engines ----
        wo_pending = []   # (tt, ec) tiles whose aT inputs are ready
        wo_state = {"cur": None, "wp": None, "nt": 0}
        norm_pending = []  # deferred (qc, h, op, sm) normalizations

        def wo_step():
            stt = wo_state
            if stt["cur"] is None:
                if not wo_pending:
                    return
                stt["cur"] = wo_pending.pop(0)
                stt["wp"] = pall.tile(
                    [128, QCH], FP32, tag="wo", bufs=2, name="wp"
                )
                stt["nt"] = 0
            tt, ec = stt["cur"]
            nt = stt["nt"]
            nc.tensor.matmul(
                stt["wp"][:],
                aTq[tt // 4][:, nt, (tt % 4) * 128 : (tt % 4 + 1) * 128],
                wo_sb[:, nt, ec * QCH : (ec + 1) * QCH],
                start=(nt == 0), stop=(nt == NH - 1),
            )
            stt["nt"] += 1
            if stt["nt"] == NH:
                ob = outpool.tile([128, QCH], BF16, tag="ob", name="ob")
                nc.vector.tensor_copy(ob[:], stt["wp"][:])
                nc.sync.dma_start(
                    out[tt * 128 : (tt + 1) * 128,
                        ec * QCH : (ec + 1) * QCH],
                    ob[:],
                )
                stt["cur"] = None

        def finish_norm():
            if not norm_pending:
                return
            qc0, h0, op0, sm0 = norm_pending.pop(0)
            rb = sbwork.tile([128, QCH], FP32, tag="rb")
            nc.gpsimd.partition_broadcast(rb[:], sm0[:])
            nc.vector.tensor_mul(aTq[qc0][:, h0, :], op0[:], rb[:])
            if h0 == NH - 1:
                wo_pending.extend(
                    (tt, ec)
                    for tt in range(qc0 * 4, (qc0 + 1) * 4)
                    for ec in range(E // QCH)
                )

        for qc in range(NQC):
            qs = slice(qc * QCH, (qc + 1) * QCH)
            for h in range(NH):
                op = pall.tile([128, QCH], FP32, tag="ot", bufs=2)

                DEPTH = 3
                pts = [None] * NKT
                l1s = [None] * (NKT // 2)
                l2s = []

                def issue_scores(kt):
                    st = pall.tile(
                        [128, QCH], FP32, tag="st", bufs=3, name="st"
                    )
                    nc.tensor.matmul(
                        st[:],
                        kT[:, kt * 128 : (kt + 1) * 128],
                        qT[:, h, qs],
                        start=True, stop=True,
                    )
                    pt = ptpool.tile([128, QCH], BF16, tag="pt", name="pt")
                    nc.scalar.activation(pt[:], st[:], exp, scale=SCALE)
                    pts[kt] = pt
                    # partition-sum prep: bf16 pair-tree on DVE (2x mode),
                    # leaving 4 partial tiles for a single PE ones-matmul
                    if kt % 2 == 1:
                        l1 = sbwork.tile(
                            [128, QCH], BF16, tag="l1", bufs=3, name="l1"
                        )
                        nc.vector.tensor_add(l1[:], pts[kt - 1][:], pt[:])
                        l1s[kt // 2] = l1
                        if kt % 4 == 3:
                            l2 = sbwork.tile(
                                [128, QCH], BF16, tag="l2", bufs=5, name="l2"
                            )
                            nc.vector.tensor_add(
                                l2[:], l1s[kt // 2 - 1][:], l1[:]
                            )
                            l2s.append(l2)

                for kt in range(DEPTH):
                    issue_scores(kt)
                    wo_step()
                for kt in range(NKT):
                    if kt + DEPTH < NKT:
                        issue_scores(kt + DEPTH)
                    if kt == 2:
                        finish_norm()
                    nc.tensor.matmul(
                        op[:], vN[:, kt, :], pts[kt][:],
                        start=(kt == 0), stop=(kt == NKT - 1),
                    )
                    wo_step()
                # partition-reduce the 4 partials -> Z, then defer the rest
                sp = pall.tile([1, QCH], FP32, tag="sm", bufs=1, name="sp")
                for i, l2 in enumerate(l2s):
                    nc.tensor.matmul(
                        sp[:], ones_bf[:], l2[:],
                        start=(i == 0), stop=(i == len(l2s) - 1),
                    )
                sm = sbwork.tile([1, QCH], FP32, tag="sm2")
                nc.vector.reciprocal(sm[:], sp[:])
                norm_pending.append((qc, h, op, sm))

        finish_norm()
        while wo_pending or wo_state["cur"] is not None:
            wo_step()


_NC_CACHE = []


def _get_nc():
    if not _NC_CACHE:
        _NC_CACHE.append(_build_core_program())
    return _NC_CACHE[0]


def _make_in_maps(inputs_q, inputs_kv, Wq, Wk, Wv, Wo):
    bf = lambda a: np.ascontiguousarray(a).astype(ml_dtypes.bfloat16)
    in_maps = []
    for core in range(8):
        b, g = core // 4, core % 4
        in_maps.append(
            {
                "xq": bf(inputs_q[b]),
                "xkv": bf(inputs_kv[b]),
                "wq": bf(Wq[:, g * ND : (g + 1) * ND]),
                "wk": bf(Wk[:, g * D : (g + 1) * D]),
                "wv": bf(Wv[:, g * D : (g + 1) * D]),
                "wo": bf(Wo[g * ND : (g + 1) * ND, :]),
            }
        )
    return in_maps


def _run(inputs_q, inputs_kv, Wq, Wk, Wv, Wo, trace=False, **trace_kwargs):
    nc = _get_nc()
    in_maps = _make_in_maps(inputs_q, inputs_kv, Wq, Wk, Wv, Wo)
    res = run_bass_kernel_spmd(
        nc, in_maps, core_ids=list(range(8)), trace=trace, **trace_kwargs
    )
    parts = [np.asarray(r["out"], dtype=np.float32) for r in res.results]
    full = np.stack(
        [
            parts[0] + parts[1] + parts[2] + parts[3],
            parts[4] + parts[5] + parts[6] + parts[7],
        ]
    ).astype(np.float32)
    return full, res


def kernel(inputs_q, inputs_kv, Wq, Wk, Wv, Wo, mask=None):
    inputs_q = np.asarray(inputs_q, dtype=np.float32)
    inputs_kv = np.asarray(inputs_kv, dtype=np.float32)
    Wq = np.asarray(Wq, dtype=np.float32)
    Wk = np.asarray(Wk, dtype=np.float32)
    Wv = np.asarray(Wv, dtype=np.float32)
    Wo = np.asarray(Wo, dtype=np.float32)
    full, _ = _run(inputs_q, inputs_kv, Wq, Wk, Wv, Wo, trace=False)
    return full
